# revision 1
# baseline (speedup 1.0000x reference)
"""KernelCRPS loss on 8 Trainium2 NeuronCores (Bass/Tile).

Math: for each grid point with ensemble p_0..p_15 and target t,
  kcrps = [ mean_k |t - p_k|  - 1/(2*E^2) * sum_{i,j} |p_i - p_j| ] * scale_v * w_p
summed over all points, divided by (sum(w) * batch).

The host prescales yh = fp16(g*y), th = fp16(g*t) with g = scale_v * w_p >= 0,
so the device only evaluates the 136 "pair rows" per point:
  120 pair rows  (i, i+d), d=1..15   -> sum_points |yh_i - yh_{i+d}|
   16 mae  rows  k                   -> sum_points |th - yh_k|
Work is split between a PE (matmul) stream and three SBUF engine paths
(rates from the TRN2 cost model; GPSIMD ucode only implements
add/subtract/mult/copy, so it can only act as a subtract producer):
  PE  a fixed {0,+-1} (17, 128) weight matrix turns each moving column
      (16 ensemble values + th) into 112 pair diffs + 16 mae diffs in
      PSUM; ScalarE Abs+accum reduces 4-bank groups.  The per-partition
      accumulator column separates pair rows from mae rows for free.
  pB  GPSIMD TT subtract (1.98 ns/elem) -> ScalarE Abs+accum (0.86)
  pD  DVE TT max @2x (0.56) + DVE ts sum-accum @4x (0.23)   (max identity)
  pA  DVE TT subtract @2x (0.56) -> ScalarE Abs+accum (0.86)
Max-identity (pD) rows use |a-b| = 2*max(a,b) - (a+b); the linear term is an
exact fp64 host-side correction from per-ensemble column sums of yh over the
SBUF point range.

Sharding: latlon 40320 -> 8 cores x 5040 (pointwise per grid point, no
cross-core math except the host-side sum of per-core partial sums).
"""

import os

import numpy as np

B, V, P, E = 2, 16, 40320, 16
NCORES = 8
PC = P // NCORES            # 5040 latlon points per core
NPT = B * V * PC            # 161280 (b, v, p) points per core
PART = 128
FREE = NPT // PART          # 1260 points per partition
PE_GROUP = int(os.environ.get("KCRPS_PE_GROUP", "2048"))  # PSUM cols per consumer instr

_CACHE = {}
LAST_EXEC_NS = None
LAST_NC = None


def _pe_w():
    """Point-columns per partition routed through the PE matmul path
    (multiple of 16; 0 disables the PE path)."""
    w = int(os.environ.get("KCRPS_PE_W", "544"))
    assert w % 16 == 0 and 0 <= w < FREE
    return w


# The 8 pair rows dropped from the 128-row PE matrix (PSUM has 128
# partitions; 120 pairs + 16 mae = 136 > 128).  Their planes for the PE
# point range arrive as a packed 7-plane chunk and are evaluated with the
# subtract+Abs path.
PE_DROP = [(12, 0), (12, 1), (13, 0), (13, 1), (13, 2),
           (14, 0), (14, 1), (15, 0)]
PE_DROP_PLANES = [0, 1, 2, 12, 13, 14, 15]
# matrix pair rows: all (d, i) except PE_DROP; then 16 mae rows
PE_PAIRS = [(d, i) for d in range(1, E) for i in range(E - d)
            if (d, i) not in PE_DROP]
assert len(PE_PAIRS) == 112


def _chunks():
    v = os.environ.get("KCRPS_CHUNKS", "")
    sbuf_free = FREE - _pe_w()
    if not v:
        if sbuf_free == 716:
            return [40, 72, 104, 136, 168, 196]
        if sbuf_free == 748:
            return [40, 72, 108, 144, 180, 204]
        if sbuf_free == 908:
            return [44, 88, 132, 180, 216, 248]
        if sbuf_free == 924:
            return [88, 132, 176, 232, 296]
        base = [64, 128, 192, 232, 256, 288]
        tot = sum(base)
        ws = [max(16, w * sbuf_free // tot) for w in base]
        ws[-1] += sbuf_free - sum(ws)
        return ws
    ws = [int(x) for x in v.split(",") if x.strip()]
    assert sum(ws) == sbuf_free, f"chunk widths must sum to {sbuf_free}"
    return ws


def _act_split():
    return int(os.environ.get("KCRPS_ACT_SPLIT", "2"))


def _mae_path():
    # "pa": |th - yh| via DVE subtract + ACT Abs; "pd": max(th, yh) via DVE
    # TT max + ts sum-accum with host-side linear correction.
    v = os.environ.get("KCRPS_MAE", "pd")
    assert v in ("pa", "pd")
    return v


def _row_split():
    """Per offset d=1..15: (pool_rows, pd_rows, pa_rows), consecutive i-ranges
    starting at i=0, summing to 16-d."""
    pool = os.environ.get("KCRPS_POOL", "15,14,13,6,0,0,0,0,0,0,0,0,0,0,0")
    pd = os.environ.get("KCRPS_PD", "0,0,0,6,11,10,9,8,7,6,5,4,3,2,1")
    pool = [int(x) for x in pool.split(",")]
    pd = [int(x) for x in pd.split(",")]
    assert len(pool) == 15 and len(pd) == 15
    split = []
    for d in range(1, E):
        n = E - d
        po, pq = pool[d - 1], pd[d - 1]
        assert po + pq <= n, f"d={d}: pool+pd rows {po}+{pq} > {n}"
        split.append((po, pq, n - po - pq))
    return split


def _build_nc(chunk_ws, split, act_split, mae_path, pe_w):
    import concourse.bacc as bacc
    from concourse import mybir, tile
    from concourse.mybir import AluOpType

    f16 = mybir.dt.float16
    f32 = mybir.dt.float32
    u16 = mybir.dt.uint16

    pb_rows = sum(po for po, _, _ in split)
    pd_rows = sum(pq for _, pq, _ in split)
    pa_rows = sum(pa for _, _, pa in split)
    n_abs = min(act_split, pa_rows) if pa_rows else 0
    n_abs_pb = min(act_split, pb_rows) if pb_rows else 0
    cols_per_chunk = (n_abs_pb + (1 if pd_rows else 0) + n_abs + 1)
    n_pe_groups = PART * pe_w // PE_GROUP if pe_w else 0
    n_drop = len(PE_DROP_PLANES)
    ncol = (cols_per_chunk * len(chunk_ws) + n_pe_groups
            + (1 if pe_w else 0))
    sbuf_free = FREE - pe_w
    y_cols = sbuf_free * E + (n_drop * pe_w if pe_w else 0)

    nc = bacc.Bacc(
        "TRN2",
        target_bir_lowering=False,
        debug=False,
        enable_asserts=False,
        num_devices=NCORES,
    )
    y = nc.dram_tensor("y", [PART, y_cols], f16, kind="ExternalInput")
    t = nc.dram_tensor("t", [PART, FREE], f16, kind="ExternalInput")
    if pe_w:
        wd = nc.dram_tensor("wm", [E + 1, PART], f16, kind="ExternalInput")
        mv = nc.dram_tensor("mv", [E + 1, PART * pe_w], f16,
                            kind="ExternalInput")
    out = nc.dram_tensor("acc", [PART, ncol], f32, kind="ExternalOutput")

    with tile.TileContext(nc) as tc:
        with (
            tc.tile_pool(name="y_pool", bufs=3) as y_pool,
            tc.tile_pool(name="pa_pool", bufs=2) as pa_pool,
            tc.tile_pool(name="pb_pool", bufs=int(os.environ.get("KCRPS_PB_BUFS", "3"))) as pb_pool,
            tc.tile_pool(name="pd_pool", bufs=1) as pd_pool,
            tc.tile_pool(name="mv_pool", bufs=3) as mv_pool,
            tc.psum_pool(name="ps_pool", bufs=8192 // PE_GROUP // 2) as ps_pool,
            tc.tile_pool(name="fix", bufs=1) as fix,
        ):
            th = fix.tile([PART, FREE], f16)
            acc = fix.tile([PART, ncol], f32)
            nc.vector.memset(acc[:], 0.0)
            wt = None
            wt_late = os.environ.get("KCRPS_WT_LATE", "1") == "1"
            if pe_w:
                wt = fix.tile([E + 1, PART], f16)
                if not wt_late:
                    nc.sync.dma_start(out=wt[:], in_=wd.ap())

            # split pair rows into roughly-equal row groups, each reduced by
            # its own ScalarE Abs+accum so ACT streams behind the producer
            # instead of waiting for the full region.
            def _groups(total, n):
                res, s = [], 0
                for g in range(n):
                    r = total // n + (1 if g < total % n else 0)
                    res.append((s, s + r))
                    s += r
                return res

            abs_groups = _groups(pa_rows, n_abs) if pa_rows else []
            pb_groups = _groups(pb_rows, n_abs_pb) if pb_rows else []

            col = 0
            pe_col = cols_per_chunk * len(chunk_ws)
            drop_col = pe_col + n_pe_groups
            pe_state = {"next": 0, "col": pe_col}
            pb_pending = []

            def emit_pe_groups(n):
                """Emit n PE (matmul stream) groups: DMA a (17, PE_GROUP)
                moving tile, 4 matmuls into a 4-bank PSUM tile, one ScalarE
                Abs+accum over the group."""
                for _ in range(n):
                    g = pe_state["next"]
                    if g >= n_pe_groups:
                        return
                    pe_state["next"] += 1
                    mt = mv_pool.tile([E + 1, PE_GROUP], f16, tag="mv")
                    _mv = os.environ.get("KCRPS_MV_ENG", "sync")
                    mv_eng = {"sync": nc.sync, "scalar": nc.scalar,
                              "gpsimd": nc.gpsimd,
                              "vector": nc.vector}[_mv]
                    mv_eng.dma_start(
                        out=mt[:],
                        in_=mv.ap()[:, g * PE_GROUP:(g + 1) * PE_GROUP])
                    pt = ps_pool.tile([PART, PE_GROUP], f32, tag="ps")
                    for q in range(PE_GROUP // 512):
                        nc.tensor.matmul(
                            out=pt[:, q * 512:(q + 1) * 512],
                            lhsT=wt[:],
                            rhs=mt[:, q * 512:(q + 1) * 512],
                            start=True, stop=True)
                    nc.scalar.activation(
                        out=pt[:], in_=pt[:],
                        func=mybir.ActivationFunctionType.Abs,
                        accum_out=acc[:, pe_state["col"]:pe_state["col"] + 1])
                    pe_state["col"] += 1

            dist_env = os.environ.get("KCRPS_PE_DIST", "")
            if dist_env and pe_w:
                pe_dist = [int(x) for x in dist_env.split(",")]
                assert len(pe_dist) == len(chunk_ws)
                assert sum(pe_dist) >= n_pe_groups
            else:
                per = ((n_pe_groups + len(chunk_ws) - 1)
                       // len(chunk_ws)) if pe_w else 0
                pe_dist = [per] * len(chunk_ws)

            off = 0
            for ci, w in enumerate(chunk_ws):
                yt = y_pool.tile([PART, E * w], f16)
                nc.sync.dma_start(
                    out=yt[:], in_=y.ap()[:, off * E:(off + w) * E])
                yv = yt[:].rearrange("p (e f) -> p e f", e=E)
                if ci == 0:
                    # chunk 0's data DMA goes first on the queue; weights +
                    # th (small, needed later) follow so they don't delay
                    # the first compute.
                    if pe_w and wt_late:
                        _wv = os.environ.get("KCRPS_MV_ENG", "sync")
                        wt_eng = {"sync": nc.sync, "scalar": nc.scalar,
                                  "gpsimd": nc.gpsimd,
                                  "vector": nc.vector}[_wv]
                        wt_eng.dma_start(out=wt[:], in_=wd.ap())
                    n_warm = int(os.environ.get("KCRPS_PE_WARM", "0"))
                    if pe_w and n_warm:
                        # dummy matmuls on the weights tile: ramps the PE
                        # out of its cold p-state during the pipeline fill
                        # so the first real PSUM group lands sooner.
                        wp = ps_pool.tile([PART, PE_GROUP], f32, tag="ps")
                        for _ in range(n_warm):
                            nc.tensor.matmul(
                                out=wp[:, 0:PART], lhsT=wt[:],
                                rhs=wt[:], start=True, stop=True)
                    emit_pe_groups(int(os.environ.get("KCRPS_EARLY_PE", "0")))
                    nc.sync.dma_start(out=th[:], in_=t.ap())

                # --- Pool path: TT subtract planes into pb scratch ---------
                # (GPSIMD ucode has no max; subtract is its only useful op.
                # e-major planes make every consecutive plane range a flat
                # 2D slice, which Pool requires.)
                pbt = None
                if pb_rows:
                    pbt = pb_pool.tile([PART, pb_rows * w], f16, tag="pb")
                    cur = 0
                    for d in range(1, E):
                        po = split[d - 1][0]
                        if po == 0:
                            continue
                        nc.gpsimd.tensor_tensor(
                            pbt[:, cur * w:(cur + po) * w],
                            yt[:, 0:po * w],
                            yt[:, d * w:(d + po) * w],
                            AluOpType.subtract)
                        cur += po

                pat = pa_pool.tile([PART, (pa_rows + E) * w], f16, tag="pa")
                pav = pat[:].rearrange("p (e f) -> p e f", f=w)
                tb = (th[:, pe_w + off:pe_w + off + w]
                      .unsqueeze(1).broadcast_to([PART, E, w]))

                if mae_path == "pa":
                    # --- mae rows first: |th - yh_k| so ACT starts early ---
                    nc.vector.tensor_tensor(
                        pav[:, pa_rows:pa_rows + E, :], yv[:, 0:E, :], tb,
                        AluOpType.subtract)
                    nc.scalar.activation(
                        out=pat[:, pa_rows * w:(pa_rows + E) * w],
                        in_=pat[:, pa_rows * w:(pa_rows + E) * w],
                        func=mybir.ActivationFunctionType.Abs,
                        accum_out=acc[:, col:col + 1])
                    col += 1

                # --- ACT path: TT subtract planes, grouped Abs+accum -------
                cur = 0
                gi = 0
                for d in range(1, E):
                    po, pq, pa = split[d - 1]
                    if pa == 0:
                        continue
                    s = po + pq
                    nc.vector.tensor_tensor(
                        pat[:, cur * w:(cur + pa) * w],
                        yt[:, s * w:(s + pa) * w],
                        yt[:, (s + d) * w:(s + d + pa) * w],
                        AluOpType.subtract)
                    cur += pa
                    while gi < len(abs_groups) and abs_groups[gi][1] <= cur:
                        g0, g1 = abs_groups[gi]
                        nc.scalar.activation(
                            out=pat[:, g0 * w:g1 * w],
                            in_=pat[:, g0 * w:g1 * w],
                            func=mybir.ActivationFunctionType.Abs,
                            accum_out=acc[:, col:col + 1])
                        col += 1
                        gi += 1

                def consume_pb():
                    # DVE reduce of the previous chunk's Pool diff planes:
                    # |x| via u16 sign-bit mask at 4x, then ts sum-accum.
                    for pbt_p, w_p, cols_p in pb_pending:
                        for (g0, g1), cc in cols_p:
                            seg16 = pbt_p[:, g0 * w_p:g1 * w_p]
                            nc.vector.tensor_scalar(
                                out=seg16.bitcast(u16),
                                in0=seg16.bitcast(u16),
                                scalar1=0x7FFF, scalar2=0,
                                op0=AluOpType.bitwise_and,
                                op1=AluOpType.bitwise_or)
                            nc.vector.tensor_scalar(
                                out=seg16, in0=seg16,
                                scalar1=0.0, scalar2=0.0,
                                op0=AluOpType.bypass, op1=AluOpType.add,
                                accum_out=acc[:, cc:cc + 1])
                    pb_pending.clear()

                if os.environ.get("KCRPS_PB_FIRST", "0") == "1":
                    consume_pb()

                # --- DVE two-pass path: TT max then ts sum-accum @4x -------
                mae_pd = E if mae_path == "pd" else 0
                if pd_rows or mae_pd:
                    pdt = pd_pool.tile(
                        [PART, (pd_rows + mae_pd) * w], f16, tag="pd")
                    cur = 0
                    for d in range(1, E):
                        po, pq, _ = split[d - 1]
                        if pq == 0:
                            continue
                        nc.vector.tensor_tensor(
                            pdt[:, cur * w:(cur + pq) * w],
                            yt[:, po * w:(po + pq) * w],
                            yt[:, (po + d) * w:(po + d + pq) * w],
                            AluOpType.max)
                        cur += pq
                    if mae_pd:
                        pdv = pdt[:].rearrange("p (e f) -> p e f", f=w)
                        nc.vector.tensor_tensor(
                            pdv[:, pd_rows:pd_rows + E, :], yv[:, 0:E, :],
                            tb, AluOpType.max)
                    if pd_rows:
                        nc.vector.tensor_scalar(
                            out=pdt[:, 0:pd_rows * w],
                            in0=pdt[:, 0:pd_rows * w],
                            scalar1=0.0, scalar2=0.0,
                            op0=AluOpType.bypass, op1=AluOpType.add,
                            accum_out=acc[:, col:col + 1])
                        col += 1
                    if mae_pd:
                        nc.vector.tensor_scalar(
                            out=pdt[:, pd_rows * w:(pd_rows + E) * w],
                            in0=pdt[:, pd_rows * w:(pd_rows + E) * w],
                            scalar1=0.0, scalar2=0.0,
                            op0=AluOpType.bypass, op1=AluOpType.add,
                            accum_out=acc[:, col:col + 1])
                        col += 1

                if os.environ.get("KCRPS_PB_FIRST", "0") != "1":
                    consume_pb()
                if pb_rows:
                    cols_p = []
                    for g in pb_groups:
                        cols_p.append((g, col))
                        col += 1
                    pb_pending.append((pbt, w, cols_p))

                emit_pe_groups(pe_dist[ci])

                if ci == int(os.environ.get("KCRPS_DROP_CHUNK", "2")) and pe_w:
                    # the 8 pair rows the PE matrix could not hold, over the
                    # PE point range: DVE subtract + one ScalarE Abs+accum.
                    # Emitted early so it does not straggle at the tail.
                    dt_ = pa_pool.tile([PART, len(PE_DROP) * pe_w], f16,
                                       tag="dr")
                    ydt = y_pool.tile([PART, n_drop * pe_w], f16,
                                      tag="ydrop")
                    nc.sync.dma_start(
                        out=ydt[:], in_=y.ap()[:, sbuf_free * E:y_cols])
                    # packed plane order PE_DROP_PLANES = [0,1,2,12,13,14,15]
                    # rows: d=12 i 0:2, d=13 i 0:3, d=14 i 0:2, d=15 i 0:1
                    emit = [
                        (2, 0, 3),   # d=12: in0 plane idx 0..1, in1 3..4
                        (3, 0, 4),   # d=13: idx 0..2 vs 4..6
                        (2, 0, 5),   # d=14: idx 0..1 vs 5..6
                        (1, 0, 6),   # d=15: idx 0 vs 6
                    ]
                    drop_eng = (nc.gpsimd
                                if os.environ.get("KCRPS_DROP_POOL", "0")
                                == "1" else nc.vector)
                    cur = 0
                    for r, i0, i1 in emit:
                        drop_eng.tensor_tensor(
                            dt_[:, cur * pe_w:(cur + r) * pe_w],
                            ydt[:, i0 * pe_w:(i0 + r) * pe_w],
                            ydt[:, i1 * pe_w:(i1 + r) * pe_w],
                            AluOpType.subtract)
                        cur += r
                    if os.environ.get("KCRPS_DROP_DVE", "0") == "1":
                        nc.vector.tensor_scalar(
                            out=dt_[:].bitcast(u16), in0=dt_[:].bitcast(u16),
                            scalar1=0x7FFF, scalar2=0,
                            op0=AluOpType.bitwise_and,
                            op1=AluOpType.bitwise_or)
                        nc.vector.tensor_scalar(
                            out=dt_[:], in0=dt_[:], scalar1=0.0, scalar2=0.0,
                            op0=AluOpType.bypass, op1=AluOpType.add,
                            accum_out=acc[:, drop_col:drop_col + 1])
                    else:
                        nc.scalar.activation(
                            out=dt_[:], in_=dt_[:],
                            func=mybir.ActivationFunctionType.Abs,
                            accum_out=acc[:, drop_col:drop_col + 1])
                off += w

            for pbt_p, w_p, cols_p in pb_pending:
                for (g0, g1), cc in cols_p:
                    seg16 = pbt_p[:, g0 * w_p:g1 * w_p]
                    nc.vector.tensor_scalar(
                        out=seg16.bitcast(u16), in0=seg16.bitcast(u16),
                        scalar1=0x7FFF, scalar2=0,
                        op0=AluOpType.bitwise_and, op1=AluOpType.bitwise_or)
                    nc.vector.tensor_scalar(
                        out=seg16, in0=seg16, scalar1=0.0, scalar2=0.0,
                        op0=AluOpType.bypass, op1=AluOpType.add,
                        accum_out=acc[:, cc:cc + 1])
            pb_pending.clear()
            emit_pe_groups(n_pe_groups - pe_state["next"])

            nc.sync.dma_start(out=out.ap(), in_=acc[:])
    nc.compile()
    nc._kcrps_meta = (chunk_ws, split, cols_per_chunk, ncol)
    return nc


def _col_kinds(chunk_ws, split, act_split, mae_path):
    """Per-SBUF-chunk accumulator column kinds, in emission order."""
    kinds = ["mae"] if mae_path == "pa" else []
    pa_rows = sum(pa for _, _, pa in split)
    if pa_rows:
        kinds.extend(["abs"] * min(act_split, pa_rows))
    if sum(pq for _, pq, _ in split) > 0:
        kinds.append("max")
    if mae_path == "pd":
        kinds.append("maemax")
    pb_rows = sum(po for po, _, _ in split)
    if pb_rows:
        kinds.extend(["abs"] * min(act_split, pb_rows))
    return kinds


def kernel(y_pred, y_target, weights, scale):
    global LAST_EXEC_NS, LAST_NC
    from concourse.bass_utils import run_bass_kernel_spmd

    pe_w = _pe_w()
    chunk_ws = _chunks()
    split = tuple(_row_split())
    act_split = _act_split()
    mae_path = _mae_path()
    key = ("nc3", tuple(chunk_ws), split, act_split, mae_path, pe_w)
    if key not in _CACHE:
        _CACHE[key] = _build_nc(chunk_ws, split, act_split, mae_path, pe_w)
    nc = _CACHE[key]
    LAST_NC = nc

    y_pred = np.asarray(y_pred, dtype=np.float32)
    y_target = np.asarray(y_target, dtype=np.float32)
    weights = np.asarray(weights, dtype=np.float32)
    scale = np.asarray(scale, dtype=np.float32)

    ghat = (scale[None, :, None] * weights[None, None, :])     # (1, V, P) f32
    yh = (y_pred * ghat[..., None]).astype(np.float16)         # (B, V, P, E)
    th = (y_target * ghat).astype(np.float16)                  # (B, V, P)

    sbuf_free = FREE - pe_w
    n_drop = len(PE_DROP_PLANES)

    # PE weight matrix: moving rows = 16 ensemble members + th
    if pe_w:
        W = np.zeros((E + 1, PART), np.float16)
        for m, (d, i) in enumerate(PE_PAIRS):
            W[i, m] = 1.0
            W[i + d, m] = -1.0
        for k in range(E):
            W[E, 112 + k] = 1.0
            W[k, 112 + k] = -1.0

    in_maps = []
    C_sbuf = np.zeros(E, np.float64)
    T1_sbuf = 0.0
    for c in range(NCORES):
        sl = slice(c * PC, (c + 1) * PC)
        arr = yh[:, :, sl, :].reshape(PART, FREE, E)
        tharr = th[:, :, sl].reshape(PART, FREE)
        segs = []
        off = pe_w
        for w in chunk_ws:
            seg = arr[:, off:off + w, :].transpose(0, 2, 1)    # (PART, E, w)
            segs.append(seg.reshape(PART, E * w))
            off += w
        imap = {}
        if pe_w:
            dseg = (arr[:, 0:pe_w, :][:, :, PE_DROP_PLANES]
                    .transpose(0, 2, 1).reshape(PART, n_drop * pe_w))
            segs.append(dseg)
            mvy = arr[:, 0:pe_w, :].reshape(PART * pe_w, E).T  # (E, S)
            mvt = tharr[:, 0:pe_w].reshape(1, PART * pe_w)
            imap["mv"] = np.ascontiguousarray(
                np.concatenate([mvy, mvt], axis=0).astype(np.float16))
            imap["wm"] = W
        imap["y"] = np.ascontiguousarray(np.concatenate(segs, axis=1))
        imap["t"] = np.ascontiguousarray(tharr)
        in_maps.append(imap)
        C_sbuf += arr[:, pe_w:, :].astype(np.float64).sum(axis=(0, 1))
        T1_sbuf += tharr[:, pe_w:].astype(np.float64).sum()

    res = run_bass_kernel_spmd(
        nc, in_maps, core_ids=list(range(NCORES)), trace=False)
    LAST_EXEC_NS = res.exec_time_ns

    kinds = _col_kinds(chunk_ws, split, act_split, mae_path)
    n_chunk_cols = len(kinds)
    n_pe_groups = PART * pe_w // PE_GROUP if pe_w else 0
    M_max = A_abs = A_mae = M_mae = 0.0
    for c in range(NCORES):
        a = res.results[c]["acc"].astype(np.float64)
        for j in range(len(chunk_ws)):
            base = j * n_chunk_cols
            for k, kind in enumerate(kinds):
                s = a[:, base + k].sum()
                if kind == "max":
                    M_max += s
                elif kind == "abs":
                    A_abs += s
                elif kind == "maemax":
                    M_mae += s
                else:
                    A_mae += s
        if pe_w:
            pe_base = n_chunk_cols * len(chunk_ws)
            pe_cols = a[:, pe_base:pe_base + n_pe_groups]
            A_abs += pe_cols[0:112, :].sum()       # matrix pair rows
            A_mae += pe_cols[112:128, :].sum()     # matrix mae rows
            A_abs += a[:, pe_base + n_pe_groups].sum()  # dropped pair rows

    # Correction for max-identity (pd) rows over the SBUF point range:
    # sum over selected rows (d, i) of (C_i + C_{i+d}).
    L = 0.0
    for d in range(1, E):
        po, pq, _ = split[d - 1]
        for i in range(po, po + pq):
            L += C_sbuf[i] + C_sbuf[i + d]

    PAIR_total = A_abs + 2.0 * M_max - L
    if mae_path == "pd":
        # sum_k |th - yh_k| = 2*sum_k max(th, yh_k) - E*T1 - sum_e C_e
        # (over the SBUF point range only; PE-range mae rows are direct)
        MAE_total = A_mae + 2.0 * M_mae - E * T1_sbuf - C_sbuf.sum()
    else:
        MAE_total = A_mae
    npoints = weights.astype(np.float64).sum()
    result = (MAE_total / E - PAIR_total / (E * E)) / (npoints * B)
    return np.float32(result)



# revision 39
# speedup vs baseline: 1.0414x; 1.0414x over previous
"""KernelCRPS loss on 8 Trainium2 NeuronCores (Bass/Tile).

Math: for each grid point with ensemble p_0..p_15 and target t,
  kcrps = [ mean_k |t - p_k|  - 1/(2*E^2) * sum_{i,j} |p_i - p_j| ] * scale_v * w_p
summed over all points, divided by (sum(w) * batch).

The host prescales yh = fp16(g*y), th = fp16(g*t) with g = scale_v * w_p >= 0.
Per grid point the device needs 120 pair values |yh_i - yh_j| (i<j) and 16
mae values |th - yh_k|.  Points (columns) are split across three paths sized
so ACT / DVE / GPSIMD all finish together:

  PE  cols: a fixed {0,+-1} (17, 128) weight matrix turns each moving column
      (16 ensemble values + th) into 112 pair diffs + 16 mae diffs in PSUM;
      ScalarE Abs+accum reduces 4-bank groups (exact).  The 8 pair rows that
      did not fit (PSUM has 128 partitions) are evaluated over the PE range
      with DVE TT max + ts sum-accum and an exact host-side linear
      correction (|a-b| = 2 max(a,b) - (a+b)).
  GPS cols: GPSIMD computes all 120 pair diffs (15 per-offset TT subtracts,
      1.98 ns/col); DVE consumes with single-pass ts relu+accum @4x
      (|d| = 2 relu(d) - d, linear part corrected host-side).
  SORT cols: DVE sorts the 16 ensemble values with a Batcher odd-even
      network (63 comparators in 10 layers; each layer is one strided
      multi-plane TT min + TT max @2x, ping-ponging between two 16-plane
      regions), then sum_{i<j}|p_i - p_j| = sum_k (2k-15) p_(k) via 16
      ts mult+accum @4x.  No host correction needed for the pair term.

  mae for GPS/SORT cols: DVE TT max(th, y_k) + ts sum-accum with the exact
  host-side correction sum|t-y| = 2 sum max(t,y) - (16 T1 + sum C).

Sharding: latlon 40320 -> 8 cores x 5040 (pointwise per grid point; host
sums per-core partials).
"""

import os

import numpy as np

B, V, P, E = 2, 16, 40320, 16
NCORES = 8
PC = P // NCORES            # 5040 latlon points per core
NPT = B * V * PC            # 161280 (b, v, p) points per core
PART = 128
FREE = NPT // PART          # 1260 points per partition
PE_GROUP = int(os.environ.get("KCRPS_PE_GROUP", "2048"))  # PSUM cols per ACT

_CACHE = {}
LAST_EXEC_NS = None
LAST_NC = None


def _pe_w():
    w = int(os.environ.get("KCRPS_PE_W", "512"))
    assert w % 16 == 0 and 0 <= w < FREE
    return w


def _gps_ws():
    v = os.environ.get("KCRPS_GPS_W", "130,130")
    return [int(x) for x in v.split(",") if x.strip()]


def _sort_ws():
    v = os.environ.get("KCRPS_SORT_W", "")
    if v:
        return [int(x) for x in v.split(",") if x.strip()]
    rest = FREE - _pe_w() - sum(_gps_ws())
    assert rest > 0
    return [rest]


# The 8 pair rows dropped from the 128-row PE matrix (PSUM has 128
# partitions; 120 pairs + 16 mae = 136 > 128).  Their planes for the PE
# point range arrive as a packed 7-plane chunk.
PE_DROP = [(12, 0), (12, 1), (13, 0), (13, 1), (13, 2),
           (14, 0), (14, 1), (15, 0)]
PE_DROP_PLANES = [0, 1, 2, 12, 13, 14, 15]
PE_PAIRS = [(d, i) for d in range(1, E) for i in range(E - d)
            if (d, i) not in PE_DROP]
assert len(PE_PAIRS) == 112

# Batcher odd-even mergesort network for 16 inputs: 63 comparators in 10
# layers.  Each layer: (grid, i_slice, j_slice, passthrough_slices) where
# grid reshapes the 16-plane axis; slices index (outer, inner) plane dims.
# A comparator set {(i, i+d)} maps to one TT min (out=i-planes) + one TT
# max (out=j-planes); untouched planes are copied to the destination
# region with ts bypass @4x.
#   grid "16"  -> planes axis stays 1-D [16]
#   grid "2x8" -> planes viewed [2, 8] (outer stride 8)
#   grid "4x4" -> planes viewed [4, 4] (outer stride 4)
#   grid "8x2" -> planes viewed [8, 2] (outer stride 2)
S16 = (slice(None),)
BATCHER = [
    ("16",  (slice(0, 16, 2),),  (slice(1, 16, 2),),  []),
    ("4x4", (S16 + (slice(0, 2),)), (S16 + (slice(2, 4),)), []),
    ("4x4", (S16 + (slice(1, 2),)), (S16 + (slice(2, 3),)),
     [("4x4", S16 + (slice(0, 4, 3),))]),
    ("2x8", (S16 + (slice(0, 4),)), (S16 + (slice(4, 8),)), []),
    ("2x8", (S16 + (slice(2, 4),)), (S16 + (slice(4, 6),)),
     [("2x8", S16 + (slice(0, 2),)), ("2x8", S16 + (slice(6, 8),))]),
    ("2x8", (S16 + (slice(1, 7, 2),)), (S16 + (slice(2, 8, 2),)),
     [("2x8", S16 + (slice(0, 8, 7),))]),
    ("16",  (slice(0, 8),),  (slice(8, 16),), []),
    ("16",  (slice(4, 8),),  (slice(8, 12),),
     [("4x4", (slice(0, 4, 3),) + S16)]),
    ("4x4", (slice(0, 3), slice(2, 4)), (slice(1, 4), slice(0, 2)),
     [("8x2", (slice(0, 8, 7),) + S16)]),
    ("16",  (slice(1, 15, 2),), (slice(2, 16, 2),),
     [("16", (slice(0, 16, 15),))]),
]


def _check_batcher():
    """Zero-one-principle check of the BATCHER table (build-time only)."""
    import itertools
    for bits in range(0, 1 << 16, 257):  # subsampled; full check done offline
        v = np.array([(bits >> b) & 1 for b in range(16)], np.int32)
        arr = v.copy()
        for grid, isl, jsl, _ in BATCHER:
            shp = {"16": (16,), "2x8": (2, 8), "4x4": (4, 4),
                   "8x2": (8, 2)}[grid]
            g = arr.reshape(shp)
            lo = np.minimum(g[isl], g[jsl]).copy()
            hi = np.maximum(g[isl], g[jsl]).copy()
            g[isl] = lo
            g[jsl] = hi
            arr = g.reshape(16)
        assert (np.diff(arr) >= 0).all()


def _build_nc(pe_w, gps_ws, sort_ws):
    import concourse.bacc as bacc
    from concourse import mybir, tile
    from concourse.mybir import AluOpType

    f16 = mybir.dt.float16
    f32 = mybir.dt.float32

    n_drop = len(PE_DROP_PLANES)
    sbuf_cols = sum(gps_ws) + sum(sort_ws)
    assert pe_w + sbuf_cols == FREE
    y_cols = sbuf_cols * E + (n_drop * pe_w if pe_w else 0)
    n_pe_groups = PART * pe_w // PE_GROUP if pe_w else 0
    relu_g = int(os.environ.get("KCRPS_RELU_GROUPS", "3"))
    # pb plane groups per gps chunk, split at d boundaries.  Front-loaded
    # (last group small) so the final DVE consume right after GPSIMD
    # finishes is short.
    d_sizes = [E - d for d in range(1, E)]
    d_off = list(np.concatenate([[0], np.cumsum(d_sizes)]))
    d_cuts = os.environ.get("KCRPS_RELU_CUTS", "4,9")
    cuts = [int(x) for x in d_cuts.split(",") if x.strip()]
    assert len(cuts) == relu_g - 1
    bounds = [0] + [int(d_off[c]) for c in cuts] + [120]
    pb_groups = [(bounds[i], bounds[i + 1]) for i in range(relu_g)]

    # per-chunk accumulator column kinds (emission order)
    kinds_gps = ["gpsrelu"] * relu_g + ["gmae"]
    kinds_sort = ["coef%d" % k for k in range(E)] + ["smae"]
    ncol = (len(kinds_gps) * len(gps_ws) + len(kinds_sort) * len(sort_ws)
            + n_pe_groups + (1 if pe_w else 0))

    nc = bacc.Bacc(
        "TRN2",
        target_bir_lowering=False,
        debug=False,
        enable_asserts=False,
        num_devices=NCORES,
    )
    y = nc.dram_tensor("y", [PART, y_cols], f16, kind="ExternalInput")
    t = nc.dram_tensor("t", [PART, FREE], f16, kind="ExternalInput")
    if pe_w:
        wd = nc.dram_tensor("wm", [E + 1, PART], f16, kind="ExternalInput")
        mv = nc.dram_tensor("mv", [E + 1, PART * pe_w], f16,
                            kind="ExternalInput")
    out = nc.dram_tensor("acc", [PART, ncol], f32, kind="ExternalOutput")

    mv_blk = int(os.environ.get("KCRPS_MV_BLK", "2"))  # PE groups per mv DMA

    with tile.TileContext(nc) as tc:
        with (
            tc.tile_pool(name="y_pool", bufs=2) as y_pool,
            tc.tile_pool(name="ys_pool",
                         bufs=min(2, len(sort_ws))) as ys_pool,
            tc.tile_pool(name="pb_pool", bufs=2) as pb_pool,
            tc.tile_pool(name="st_pool",
                         bufs=min(2, len(sort_ws))) as st_pool,
            tc.tile_pool(name="sc_pool", bufs=2) as sc_pool,
            tc.tile_pool(name="dr_pool", bufs=1) as dr_pool,
            tc.tile_pool(name="mv_pool", bufs=3) as mv_pool,
            tc.psum_pool(name="ps_pool", bufs=8192 // PE_GROUP // 2) as ps_pool,
            tc.tile_pool(name="fix", bufs=1) as fix,
        ):
            th = fix.tile([PART, FREE], f16)
            acc = fix.tile([PART, ncol], f32)
            nc.vector.memset(acc[:], 0.0)
            wt = None
            if pe_w:
                wt = fix.tile([E + 1, PART], f16)

            col = [0]
            pe_state = {"next": 0,
                        "col": (len(kinds_gps) * len(gps_ws)
                                + len(kinds_sort) * len(sort_ws))}
            drop_col = pe_state["col"] + n_pe_groups
            mv_tiles = {}

            def emit_pe_groups(n):
                for _ in range(n):
                    g = pe_state["next"]
                    if g >= n_pe_groups:
                        return
                    pe_state["next"] += 1
                    blk = g // mv_blk
                    if blk not in mv_tiles:
                        g0 = blk * mv_blk
                        g1 = min(g0 + mv_blk, n_pe_groups)
                        mt = mv_pool.tile([E + 1, (g1 - g0) * PE_GROUP], f16,
                                          tag="mv")
                        nc.scalar.dma_start(
                            out=mt[:],
                            in_=mv.ap()[:, g0 * PE_GROUP:g1 * PE_GROUP])
                        mv_tiles[blk] = (mt, g0)
                    mt, g0 = mv_tiles[blk]
                    off = (g - g0) * PE_GROUP
                    pt = ps_pool.tile([PART, PE_GROUP], f32, tag="ps")
                    for q in range(PE_GROUP // 512):
                        nc.tensor.matmul(
                            out=pt[:, q * 512:(q + 1) * 512],
                            lhsT=wt[:],
                            rhs=mt[:, off + q * 512:off + (q + 1) * 512],
                            start=True, stop=True)
                    nc.scalar.activation(
                        out=pt[:], in_=pt[:],
                        func=mybir.ActivationFunctionType.Abs,
                        accum_out=acc[:, pe_state["col"]:pe_state["col"] + 1])
                    pe_state["col"] += 1

            # ---- DMA schedule: interleave gps and sort chunks so both
            # GPSIMD and the DVE sort start early.
            gps_off = []
            off = 0
            for w in gps_ws:
                gps_off.append(off)
                off += w
            sort_off = []
            for w in sort_ws:
                sort_off.append(off)
                off += w
            gps_tiles = [None] * len(gps_ws)
            sort_tiles = [None] * len(sort_ws)
            # build the DMA op list: sort chunks are split into two
            # 8-plane halves so the first sort layer can start after h1.
            dma_ops = []            # (kind, i, half)
            omode = os.environ.get("KCRPS_DMA_ORDER", "gs")
            gs = [("g", i, None) for i in range(len(gps_ws))]
            ss = []
            for i in range(len(sort_ws)):
                ss += [("s", i, 0), ("s", i, 1)]
            if omode == "sg":       # all sort halves, then gps
                dma_ops = ss + gs
            elif omode == "sA":     # h1, g0, h2, g1, ...
                dma_ops = []
                pool_ = ss + gs
                a, b = ss, gs
                while a or b:
                    if a:
                        dma_ops.append(a.pop(0))
                    if b:
                        dma_ops.append(b.pop(0))
            else:                   # "gs": g0, h1, h2, g1, ...
                a, b = gs, ss
                while a or b:
                    if a:
                        dma_ops.append(a.pop(0))
                    if b:
                        dma_ops.append(b.pop(0))
                    if b:
                        dma_ops.append(b.pop(0))
            if pe_w:
                nc.sync.dma_start(out=wt[:], in_=wd.ap())
            th_early = os.environ.get("KCRPS_TH_EARLY", "1") == "1"
            th_pos = int(os.environ.get("KCRPS_TH_POS", "1"))
            if th_early:
                dma_ops.insert(th_pos, ("t", 0, None))
            for kind, i, half in dma_ops:
                if kind == "t":
                    nc.sync.dma_start(out=th[:], in_=t.ap())
                    continue
                if kind == "g":
                    w, o = gps_ws[i], gps_off[i]
                    if gps_tiles[i] is None:
                        yt = y_pool.tile([PART, E * w], f16, tag="ygps")
                        gps_tiles[i] = (yt, w, o)
                    yt = gps_tiles[i][0]
                    nc.sync.dma_start(
                        out=yt[:], in_=y.ap()[:, o * E:(o + w) * E])
                else:
                    w, o = sort_ws[i], sort_off[i]
                    if sort_tiles[i] is None:
                        yt = ys_pool.tile([PART, E * w], f16, tag="ysort")
                        sort_tiles[i] = (yt, w, o)
                    yt = sort_tiles[i][0]
                    h = E // 2 * w
                    if half == 0:
                        nc.sync.dma_start(
                            out=yt[:, 0:h], in_=y.ap()[:, o * E:o * E + h])
                    else:
                        nc.sync.dma_start(
                            out=yt[:, h:2 * h],
                            in_=y.ap()[:, o * E + h:(o + w) * E])
            if not th_early:
                nc.sync.dma_start(out=th[:], in_=t.ap())
            ydt = None
            if pe_w:
                ydt = ys_pool.tile([PART, n_drop * pe_w], f16, tag="ydrop")
                nc.sync.dma_start(
                    out=ydt[:], in_=y.ap()[:, sbuf_cols * E:y_cols])

            # ---- GPSIMD: all pair diffs of gps chunks ----------------------
            pb_tiles = []
            for yt, w, off0 in gps_tiles:
                pbt = pb_pool.tile([PART, 120 * w], f16, tag="pb")
                cur = 0
                for d in range(1, E):
                    n = E - d
                    nc.gpsimd.tensor_tensor(
                        pbt[:, cur * w:(cur + n) * w],
                        yt[:, 0:n * w],
                        yt[:, d * w:(d + n) * w],
                        AluOpType.subtract)
                    cur += n
                pb_tiles.append((pbt, yt, w, off0))

            # ---- DVE program -----------------------------------------------
            # interleave: sort layers (bulk), gps relu/mae groups (as GPSIMD
            # output becomes ready), drop rows, PE groups stream on ACT.
            def grid_view(tile_ap, grid, w):
                if grid == "16":
                    return tile_ap.rearrange("p (e f) -> p e f", f=w)
                a = {"2x8": 2, "4x4": 4, "8x2": 8}[grid]
                return tile_ap.rearrange("p (a b f) -> p a b f", a=a, f=w)

            def emit_sort(yt, w):
                """Batcher sort of the 16 e-planes of yt; returns the tile
                holding the ascending sorted planes."""
                ta = st_pool.tile([PART, E * w], f16, tag="sa")
                tb = st_pool.tile([PART, E * w], f16, tag="sb")
                src, dst = None, ta  # layer 0 reads yt
                for li, (grid, isl, jsl, passes) in enumerate(BATCHER):
                    rd = yt if li == 0 else src
                    gv_r = grid_view(rd[:], grid, w)
                    gv_w = grid_view(dst[:], grid, w)
                    if li == 0:
                        # split by plane half: first half only needs the
                        # first 8-plane DMA
                        for lo, hi in ((0, 8), (8, 16)):
                            i_h = gv_r[:, lo:hi:2, :]
                            j_h = gv_r[:, lo + 1:hi:2, :]
                            nc.vector.tensor_tensor(
                                gv_w[:, lo:hi:2, :], i_h, j_h,
                                AluOpType.min)
                            nc.vector.tensor_tensor(
                                gv_w[:, lo + 1:hi:2, :], i_h, j_h,
                                AluOpType.max)
                        src, dst = dst, (tb if dst is ta else ta)
                        continue
                    i_in = gv_r[(slice(None),) + isl]
                    j_in = gv_r[(slice(None),) + jsl]
                    nc.vector.tensor_tensor(
                        gv_w[(slice(None),) + isl], i_in, j_in,
                        AluOpType.min)
                    nc.vector.tensor_tensor(
                        gv_w[(slice(None),) + jsl], i_in, j_in,
                        AluOpType.max)
                    for pgrid, psl in passes:
                        # copy via TT min(x, x): the DVE tensor_scalar
                        # lowering rejects strided multi-dim APs.
                        pv_r = grid_view(rd[:], pgrid, w)
                        pv_w = grid_view(dst[:], pgrid, w)
                        nc.vector.tensor_tensor(
                            pv_w[(slice(None),) + psl],
                            pv_r[(slice(None),) + psl],
                            pv_r[(slice(None),) + psl],
                            AluOpType.min)
                    src, dst = dst, (tb if dst is ta else ta)
                return src, dst  # sorted tile, free scratch tile

            def emit_drop():
                dt_ = dr_pool.tile([PART, len(PE_DROP) * pe_w], f16,
                                   tag="dr")
                emit = [
                    (2, 0, 3),   # d=12: planes idx 0..1 vs 3..4
                    (3, 0, 4),   # d=13: idx 0..2 vs 4..6
                    (2, 0, 5),   # d=14: idx 0..1 vs 5..6
                    (1, 0, 6),   # d=15: idx 0 vs 6
                ]
                cur = 0
                for r, i0, i1 in emit:
                    nc.vector.tensor_tensor(
                        dt_[:, cur * pe_w:(cur + r) * pe_w],
                        ydt[:, i0 * pe_w:(i0 + r) * pe_w],
                        ydt[:, i1 * pe_w:(i1 + r) * pe_w],
                        AluOpType.max)
                    cur += r
                nc.vector.tensor_scalar(
                    out=dt_[:], in0=dt_[:], scalar1=0.0, scalar2=0.0,
                    op0=AluOpType.bypass, op1=AluOpType.add,
                    accum_out=acc[:, drop_col:drop_col + 1])

            # --- interleaved emission --------------------------------------
            # Column order must match host decode: per gps chunk
            # [relu x relu_g, gmae], then per sort chunk [coef x16, smae];
            # emission order differs, so allocate columns up-front.
            col_map = {}
            c = 0
            for gi in range(len(gps_ws)):
                for g in range(relu_g):
                    col_map[("gpsrelu", gi, g)] = c
                    c += 1
                col_map[("gmae", gi)] = c
                c += 1
            for si in range(len(sort_ws)):
                for k in range(E):
                    col_map[("coef", si, k)] = c
                    c += 1
                col_map[("smae", si)] = c
                c += 1
            assert c == pe_state["col"]

            def gps_consume(gi, g, scratch=None):
                # The elementwise relu output is unused (only accum_out
                # matters).  Writing it into the sort scratch region gives
                # the op a WAR hazard against the final sort layers, which
                # pins it late in the DVE stream -- the tile scheduler's
                # internal cost model underestimates GPSIMD time by ~2.4x
                # and otherwise hoists these between early sort layers,
                # head-of-line blocking the DVE for many microseconds.
                pbt, yt, w, off0 = pb_tiles[gi]
                g0, g1 = pb_groups[g]
                cc = col_map[("gpsrelu", gi, g)]
                n = (g1 - g0) * w
                out_ap = (scratch[:, 0:n] if scratch is not None
                          else pbt[:, g0 * w:g1 * w])
                nc.vector.tensor_scalar(
                    out=out_ap, in0=pbt[:, g0 * w:g1 * w],
                    scalar1=0.0, scalar2=0.0,
                    op0=AluOpType.max, op1=AluOpType.add,
                    accum_out=acc[:, cc:cc + 1])

            def gps_mae(gi):
                pbt, yt, w, off0 = pb_tiles[gi]
                mt = sc_pool.tile([PART, E * w], f16, tag="gmae")
                yv = yt[:].rearrange("p (e f) -> p e f", e=E)
                tb = (th[:, pe_w + off0:pe_w + off0 + w]
                      .unsqueeze(1).broadcast_to([PART, E, w]))
                mv_ = mt[:].rearrange("p (e f) -> p e f", e=E)
                nc.vector.tensor_tensor(mv_[:, :, :], yv[:, :, :], tb,
                                        AluOpType.max)
                cc = col_map[("gmae", gi)]
                nc.vector.tensor_scalar(
                    out=mt[:], in0=mt[:], scalar1=0.0, scalar2=0.0,
                    op0=AluOpType.bypass, op1=AluOpType.add,
                    accum_out=acc[:, cc:cc + 1])

            def sort_coef(si, srt, w):
                sv = srt[:].rearrange("p (e f) -> p e f", e=E)
                for k in range(E):
                    cc = col_map[("coef", si, k)]
                    nc.vector.tensor_scalar(
                        out=sv[:, k, :], in0=sv[:, k, :],
                        scalar1=float(2 * k - (E - 1)), scalar2=0.0,
                        op0=AluOpType.mult, op1=AluOpType.add,
                        accum_out=acc[:, cc:cc + 1])

            def sort_mae(si, srt, scratch, w, off0):
                sv = srt[:].rearrange("p (e f) -> p e f", e=E)
                tb = (th[:, pe_w + off0:pe_w + off0 + w]
                      .unsqueeze(1).broadcast_to([PART, E, w]))
                mv_ = scratch[:].rearrange("p (e f) -> p e f", e=E)
                nc.vector.tensor_tensor(mv_[:, :, :], sv[:, :, :], tb,
                                        AluOpType.max)
                cc = col_map[("smae", si)]
                nc.vector.tensor_scalar(
                    out=scratch[:], in0=scratch[:], scalar1=0.0,
                    scalar2=0.0,
                    op0=AluOpType.bypass, op1=AluOpType.add,
                    accum_out=acc[:, cc:cc + 1])

            # emission: interleave DVE work so it rarely stalls on GPSIMD,
            # and spread PE-group emission so mv DMA keeps ahead of PE.
            ngps = len(gps_tiles)
            assert len(sort_tiles) >= 1
            # kick a first batch of PE groups so ACT starts early
            emit_pe_groups(int(os.environ.get("KCRPS_EARLY_PE", "4")))

            # sort chunks at high priority: the scheduler slots gps
            # consumers into DVE idle moments but prefers sort work the
            # moment its data lands.
            scratches = []
            for si, (yts, ws, offs) in enumerate(sort_tiles):
                with tc.high_priority():
                    srt, scr = emit_sort(yts, ws)
                    sort_mae(si, srt, scr, ws, offs)
                    sort_coef(si, srt, ws)
                scratches.append(scr)
                emit_pe_groups(4)
            if pe_w:
                emit_drop()

            # gps consumers last; their dummy outputs write into the final
            # sort scratch to pin them after the sort (see gps_consume).
            pin = scratches[-1]
            for gi in range(ngps):
                for g in range(relu_g):
                    gps_consume(gi, g, scratch=pin)
                    emit_pe_groups(2)
                gps_mae(gi)

            emit_pe_groups(n_pe_groups - pe_state["next"])

            nc.sync.dma_start(out=out.ap(), in_=acc[:])
    nc.compile()
    nc._kcrps_meta = (pe_w, tuple(gps_ws), tuple(sort_ws), relu_g, ncol)
    return nc


def kernel(y_pred, y_target, weights, scale):
    global LAST_EXEC_NS, LAST_NC
    from concourse.bass_utils import run_bass_kernel_spmd

    pe_w = _pe_w()
    gps_ws = _gps_ws()
    sort_ws = _sort_ws()
    relu_g = int(os.environ.get("KCRPS_RELU_GROUPS", "3"))
    key = ("v2", pe_w, tuple(gps_ws), tuple(sort_ws), relu_g, PE_GROUP)
    if key not in _CACHE:
        _CACHE[key] = _build_nc(pe_w, gps_ws, sort_ws)
    nc = _CACHE[key]
    LAST_NC = nc

    y_pred = np.asarray(y_pred, dtype=np.float32)
    y_target = np.asarray(y_target, dtype=np.float32)
    weights = np.asarray(weights, dtype=np.float32)
    scale = np.asarray(scale, dtype=np.float32)

    ghat = (scale[None, :, None] * weights[None, None, :])     # (1, V, P)
    yh = (y_pred * ghat[..., None]).astype(np.float16)         # (B, V, P, E)
    th = (y_target * ghat).astype(np.float16)                  # (B, V, P)

    n_drop = len(PE_DROP_PLANES)
    sbuf_cols = sum(gps_ws) + sum(sort_ws)
    gps_tot = sum(gps_ws)

    if pe_w:
        W = np.zeros((E + 1, PART), np.float16)
        for m, (d, i) in enumerate(PE_PAIRS):
            W[i, m] = 1.0
            W[i + d, m] = -1.0
        for k in range(E):
            W[E, 112 + k] = 1.0
            W[k, 112 + k] = -1.0

    in_maps = []
    C_gps = np.zeros(E, np.float64)
    C_sbuf = np.zeros(E, np.float64)
    C_pe = np.zeros(E, np.float64)
    T1_sbuf = 0.0
    for c in range(NCORES):
        sl = slice(c * PC, (c + 1) * PC)
        arr = yh[:, :, sl, :].reshape(PART, FREE, E)
        tharr = th[:, :, sl].reshape(PART, FREE)
        segs = []
        off = pe_w
        for w in list(gps_ws) + list(sort_ws):
            seg = arr[:, off:off + w, :].transpose(0, 2, 1)    # (PART, E, w)
            segs.append(seg.reshape(PART, E * w))
            off += w
        imap = {}
        if pe_w:
            dseg = (arr[:, 0:pe_w, :][:, :, PE_DROP_PLANES]
                    .transpose(0, 2, 1).reshape(PART, n_drop * pe_w))
            segs.append(dseg)
            mvy = arr[:, 0:pe_w, :].reshape(PART * pe_w, E).T  # (E, S)
            mvt = tharr[:, 0:pe_w].reshape(1, PART * pe_w)
            imap["mv"] = np.ascontiguousarray(
                np.concatenate([mvy, mvt], axis=0).astype(np.float16))
            imap["wm"] = W
            C_pe += arr[:, 0:pe_w, :].astype(np.float64).sum(axis=(0, 1))
        imap["y"] = np.ascontiguousarray(np.concatenate(segs, axis=1))
        imap["t"] = np.ascontiguousarray(tharr)
        in_maps.append(imap)
        C_gps += (arr[:, pe_w:pe_w + gps_tot, :]
                  .astype(np.float64).sum(axis=(0, 1)))
        C_sbuf += arr[:, pe_w:, :].astype(np.float64).sum(axis=(0, 1))
        T1_sbuf += tharr[:, pe_w:].astype(np.float64).sum()

    res = run_bass_kernel_spmd(
        nc, in_maps, core_ids=list(range(NCORES)), trace=False)
    LAST_EXEC_NS = res.exec_time_ns

    n_pe_groups = PART * pe_w // PE_GROUP if pe_w else 0
    R_relu = M_gmae = M_smae = 0.0
    PAIR_sort = 0.0
    A_abs = A_mae = M_drop = 0.0
    for c in range(NCORES):
        a = res.results[c]["acc"].astype(np.float64)
        cc = 0
        for gi in range(len(gps_ws)):
            for g in range(relu_g):
                R_relu += a[:, cc].sum()
                cc += 1
            M_gmae += a[:, cc].sum()
            cc += 1
        for si in range(len(sort_ws)):
            for k in range(E):
                PAIR_sort += a[:, cc].sum()
                cc += 1
            M_smae += a[:, cc].sum()
            cc += 1
        if pe_w:
            pe_cols = a[:, cc:cc + n_pe_groups]
            A_abs += pe_cols[0:112, :].sum()       # matrix pair rows
            A_mae += pe_cols[112:128, :].sum()     # matrix mae rows
            M_drop += a[:, cc + n_pe_groups].sum()  # dropped pair rows

    # linear corrections (exact, fp64, from fp16 inputs)
    L_gps = 0.0          # sum over all (d,i) pairs of (C_i - C_{i+d})
    for d in range(1, E):
        for i in range(E - d):
            L_gps += C_gps[i] - C_gps[i + d]
    L_drop = 0.0
    for d, i in PE_DROP:
        L_drop += C_pe[i] + C_pe[i + d]

    PAIR_total = (A_abs + PAIR_sort
                  + 2.0 * R_relu - L_gps
                  + 2.0 * M_drop - L_drop)
    MAE_total = (A_mae + 2.0 * (M_gmae + M_smae)
                 - E * T1_sbuf - C_sbuf.sum())
    npoints = weights.astype(np.float64).sum()
    result = (MAE_total / E - PAIR_total / (E * E)) / (npoints * B)
    return np.float32(result)


# revision 65
# speedup vs baseline: 1.1330x; 1.0880x over previous
"""KernelCRPS loss on 8 Trainium2 NeuronCores (Bass/Tile).

Math: for each grid point with ensemble p_0..p_15 and target t,
  kcrps = [ mean_k |t - p_k|  - 1/(2*E^2) * sum_{i,j} |p_i - p_j| ] * scale_v * w_p
summed over all points, divided by (sum(w) * batch).

The host prescales yh = fp16(g*y), th = fp16(g*t) with g = scale_v * w_p >= 0.
Per grid point the device needs 120 pair values |yh_i - yh_j| (i<j) and 16
mae values |th - yh_k|.  Points (columns) are split across three paths sized
so ACT / DVE / GPSIMD all finish together:

  PE  cols: a fixed {0,+-1} (17, 128) weight matrix turns each moving column
      (16 ensemble values + th) into 112 pair diffs + 16 mae diffs in PSUM;
      ScalarE Abs+accum reduces 4-bank groups (exact).  The 8 pair rows that
      did not fit (PSUM has 128 partitions) are evaluated over the PE range
      with DVE TT max + ts sum-accum and an exact host-side linear
      correction (|a-b| = 2 max(a,b) - (a+b)).
  GPS cols: GPSIMD computes all 120 pair diffs (15 per-offset TT subtracts,
      1.98 ns/col); DVE consumes with single-pass ts relu+accum @4x
      (|d| = 2 relu(d) - d, linear part corrected host-side).
  SORT cols: DVE sorts the 16 ensemble values with a Batcher odd-even
      network (63 comparators in 10 layers; each layer is one strided
      multi-plane TT min + TT max @2x, ping-ponging between two 16-plane
      regions), then sum_{i<j}|p_i - p_j| = sum_k (2k-15) p_(k) via 16
      ts mult+accum @4x.  No host correction needed for the pair term.

  mae for GPS/SORT cols: DVE TT max(th, y_k) + ts sum-accum with the exact
  host-side correction sum|t-y| = 2 sum max(t,y) - (16 T1 + sum C).

Sharding: latlon 40320 -> 8 cores x 5040 (pointwise per grid point; host
sums per-core partials).
"""

import os

import numpy as np

B, V, P, E = 2, 16, 40320, 16
NCORES = 8
PC = P // NCORES            # 5040 latlon points per core
NPT = B * V * PC            # 161280 (b, v, p) points per core
PART = 128
FREE = NPT // PART          # 1260 points per partition
PE_GROUP = int(os.environ.get("KCRPS_PE_GROUP", "2048"))  # PSUM cols per ACT

_CACHE = {}
LAST_EXEC_NS = None
LAST_NC = None


def _pe_w():
    w = int(os.environ.get("KCRPS_PE_W", "512"))
    assert w % 16 == 0 and 0 <= w < FREE
    return w


def _gps_ws():
    v = os.environ.get("KCRPS_GPS_W", "124,124")
    return [int(x) for x in v.split(",") if x.strip()]


def _sort_ws():
    v = os.environ.get("KCRPS_SORT_W", "")
    if v:
        return [int(x) for x in v.split(",") if x.strip()]
    rest = FREE - _pe_w() - sum(_gps_ws())
    assert rest > 0
    return [rest]


# The 8 pair rows dropped from the 128-row PE matrix (PSUM has 128
# partitions; 120 pairs + 16 mae = 136 > 128).  Their planes for the PE
# point range arrive as a packed 7-plane chunk.
PE_DROP = [(12, 0), (12, 1), (13, 0), (13, 1), (13, 2),
           (14, 0), (14, 1), (15, 0)]
PE_DROP_PLANES = [0, 1, 2, 12, 13, 14, 15]
PE_PAIRS = [(d, i) for d in range(1, E) for i in range(E - d)
            if (d, i) not in PE_DROP]
assert len(PE_PAIRS) == 112

# Batcher odd-even mergesort network for 16 inputs: 63 comparators in 10
# layers.  Each layer: (grid, i_slice, j_slice, passthrough_slices) where
# grid reshapes the 16-plane axis; slices index (outer, inner) plane dims.
# A comparator set {(i, i+d)} maps to one TT min (out=i-planes) + one TT
# max (out=j-planes); untouched planes are copied to the destination
# region with ts bypass @4x.
#   grid "16"  -> planes axis stays 1-D [16]
#   grid "2x8" -> planes viewed [2, 8] (outer stride 8)
#   grid "4x4" -> planes viewed [4, 4] (outer stride 4)
#   grid "8x2" -> planes viewed [8, 2] (outer stride 2)
S16 = (slice(None),)
# raw comparator layers (d, i-list); layer 0 handled specially (reads the
# DMA-in tile in two plane halves)
BATCHER_PAIRS = [
    (1, [0, 2, 4, 6, 8, 10, 12, 14]),
    (2, [0, 1, 4, 5, 8, 9, 12, 13]),
    (1, [1, 5, 9, 13]),
    (4, [0, 1, 2, 3, 8, 9, 10, 11]),
    (2, [2, 3, 10, 11]),
    (1, [1, 3, 5, 9, 11, 13]),
    (8, [0, 1, 2, 3, 4, 5, 6, 7]),
    (4, [4, 5, 6, 7]),
    (2, [2, 3, 6, 7, 10, 11]),
    (1, [1, 3, 5, 7, 9, 11, 13]),
]

_GRIDS = {"16": (16, 1), "2x8": (2, 8), "4x4": (4, 4), "8x2": (8, 2)}


def _express(planes):
    """Find (grid, outer_slice, inner_slice) whose row-major traversal
    yields exactly `planes` (an increasing tuple)."""
    planes = tuple(planes)
    n = len(planes)
    for gname, (ga, gb) in _GRIDS.items():
        for ocnt in range(1, ga + 1):
            if n % ocnt:
                continue
            icnt = n // ocnt
            if icnt > gb:
                continue
            for o0 in range(ga):
                osteps = range(1, ga) if ocnt > 1 else (1,)
                for ostep in osteps:
                    if o0 + (ocnt - 1) * ostep >= ga:
                        continue
                    for i0 in range(gb):
                        isteps = range(1, gb) if icnt > 1 else (1,)
                        for istep in isteps:
                            if i0 + (icnt - 1) * istep >= gb:
                                continue
                            s = tuple(o * gb + i0 + k * istep
                                      for o in range(o0, o0 + ocnt * ostep,
                                                     ostep)
                                      for k in range(icnt))
                            if s == planes:
                                return (
                                    gname,
                                    slice(o0, o0 + (ocnt - 1) * ostep + 1,
                                          ostep),
                                    slice(i0, i0 + (icnt - 1) * istep + 1,
                                          istep))
    return None


def _express_or_split(planes):
    """Express `planes` as >=1 (grid, osl, isl) groups."""
    e = _express(planes)
    if e is not None:
        return [(e, tuple(planes))]
    assert len(planes) > 1, f"cannot express {planes}"
    h = len(planes) // 2
    return (_express_or_split(planes[:h])
            + _express_or_split(planes[h:]))


def _plan_scattered():
    """Plan the Batcher network with per-plane buffer tracking (A=0, B=1)
    and no passthrough copies: compared planes always write to the
    opposite buffer, untouched planes stay put.  Layer 0 reads the DMA
    tile and writes everything to A.

    Returns (layer_ops, b_final): layer_ops = list (per layer 1..9) of
    sub-ops (bi, bj, expr_i, expr_j) where expr = (grid, osl, isl) view
    slices for the i-planes / j-planes; b_final[p] = buffer of sorted
    plane p."""
    b = [0] * E          # after layer 0 everything is in A
    layer_ops = []
    for d, ilist in BATCHER_PAIRS[1:]:
        groups = {}
        for i in ilist:
            groups.setdefault((b[i], b[i + d]), []).append(i)
        ops = []
        for (bi, bj), iset in sorted(groups.items()):
            for expr_i, pl in _express_or_split(tuple(sorted(iset))):
                jpl = tuple(p + d for p in pl)
                sub = _express_or_split(jpl)
                if len(sub) == 1:
                    ops.append((bi, bj, expr_i, sub[0][0], pl, jpl))
                else:
                    # split i to match j's split granularity
                    for expr_j, jp in sub:
                        ip = tuple(p - d for p in jp)
                        ei = _express(ip)
                        assert ei is not None
                        ops.append((bi, bj, ei, expr_j, ip, jp))
        layer_ops.append(ops)
        for i in ilist:
            b[i] ^= 1
            b[i + d] ^= 1
    return layer_ops, b


def _check_scattered():
    """Zero-one-principle check of the scattered plan (all 2^16 inputs,
    vectorized)."""
    layer_ops, b_final = _plan_scattered()
    nvec = 1 << E
    vals = ((np.arange(nvec, dtype=np.uint32)[:, None]
             >> np.arange(E)[None, :]) & 1).astype(np.int8)
    A = vals.copy()          # layer 0: sorted pairs written to A
    B = np.zeros_like(A)
    for i in range(0, E, 2):
        lo = np.minimum(vals[:, i], vals[:, i + 1])
        hi = np.maximum(vals[:, i], vals[:, i + 1])
        A[:, i], A[:, i + 1] = lo, hi
    bufs = [A, B]
    for ops in layer_ops:
        writes = []
        for bi, bj, _, _, pl, jpl in ops:
            vi = bufs[bi][:, list(pl)]
            vj = bufs[bj][:, list(jpl)]
            writes.append((1 - bi, pl, np.minimum(vi, vj)))
            writes.append((1 - bj, jpl, np.maximum(vi, vj)))
        for wb, wpl, wv in writes:
            bufs[wb][:, list(wpl)] = wv
    out = np.stack([bufs[b_final[p]][:, p] for p in range(E)], axis=1)
    assert (np.diff(out, axis=1) >= 0).all(), "scattered plan does not sort"


_check_scattered()
_SCATTER_OPS, _B_FINAL = _plan_scattered()


def _build_nc(pe_w, gps_ws, sort_ws):
    import concourse.bacc as bacc
    from concourse import mybir, tile
    from concourse.mybir import AluOpType

    f16 = mybir.dt.float16
    f32 = mybir.dt.float32

    n_drop = len(PE_DROP_PLANES)
    sbuf_cols = sum(gps_ws) + sum(sort_ws)
    assert pe_w + sbuf_cols == FREE
    y_cols = sbuf_cols * E + (n_drop * pe_w if pe_w else 0)
    n_pe_groups = PART * pe_w // PE_GROUP if pe_w else 0
    relu_g = int(os.environ.get("KCRPS_RELU_GROUPS", "3"))
    # pb plane groups per gps chunk, split at d boundaries.  Front-loaded
    # (last group small) so the final DVE consume right after GPSIMD
    # finishes is short.
    d_sizes = [E - d for d in range(1, E)]
    d_off = list(np.concatenate([[0], np.cumsum(d_sizes)]))
    d_cuts = os.environ.get("KCRPS_RELU_CUTS", "4,9")
    cuts = [int(x) for x in d_cuts.split(",") if x.strip()]
    assert len(cuts) == relu_g - 1
    bounds = [0] + [int(d_off[c]) for c in cuts] + [120]
    pb_groups = [(bounds[i], bounds[i + 1]) for i in range(relu_g)]

    # per-chunk accumulator column kinds (emission order)
    kinds_gps = ["gpsrelu"] * relu_g + ["gmae"]
    kinds_sort = ["coef%d" % k for k in range(E)] + ["smae"]
    ncol = (len(kinds_gps) * len(gps_ws) + len(kinds_sort) * len(sort_ws)
            + n_pe_groups + (1 if pe_w else 0))

    nc = bacc.Bacc(
        "TRN2",
        target_bir_lowering=False,
        debug=False,
        enable_asserts=False,
        num_devices=NCORES,
    )
    y = nc.dram_tensor("y", [PART, y_cols], f16, kind="ExternalInput")
    t = nc.dram_tensor("t", [PART, FREE], f16, kind="ExternalInput")
    if pe_w:
        wd = nc.dram_tensor("wm", [E + 1, PART], f16, kind="ExternalInput")
        mv = nc.dram_tensor("mv", [E + 1, PART * pe_w], f16,
                            kind="ExternalInput")
    out = nc.dram_tensor("acc", [PART, ncol], f32, kind="ExternalOutput")

    mv_blk = int(os.environ.get("KCRPS_MV_BLK", "2"))  # PE groups per mv DMA

    with tile.TileContext(nc) as tc:
        with (
            tc.tile_pool(name="y_pool", bufs=2) as y_pool,
            tc.tile_pool(name="ys_pool",
                         bufs=min(2, len(sort_ws))) as ys_pool,
            tc.tile_pool(name="pb_pool", bufs=2) as pb_pool,
            tc.tile_pool(name="st_pool",
                         bufs=min(2, len(sort_ws))) as st_pool,
            tc.tile_pool(name="sc_pool", bufs=2) as sc_pool,
            tc.tile_pool(name="sm_pool",
                         bufs=min(2, len(sort_ws))) as sm_pool,
            tc.tile_pool(name="dr_pool", bufs=1) as dr_pool,
            tc.tile_pool(name="mv_pool", bufs=3) as mv_pool,
            tc.tile_pool(name="mv0_pool", bufs=1) as mv0_pool,
            tc.psum_pool(name="ps_pool", bufs=8192 // PE_GROUP // 2) as ps_pool,
            tc.tile_pool(name="fix", bufs=1) as fix,
        ):
            th = fix.tile([PART, FREE], f16)
            acc = fix.tile([PART, ncol], f32)
            nc.vector.memset(acc[:], 0.0)
            wt = None
            if pe_w:
                wt = fix.tile([E + 1, PART], f16)

            col = [0]
            pe_state = {"next": 0,
                        "col": (len(kinds_gps) * len(gps_ws)
                                + len(kinds_sort) * len(sort_ws))}
            drop_col = pe_state["col"] + n_pe_groups
            mv_tiles = {}

            # mv block ranges: a larger first block keeps the PE fed (and
            # p-state ramped) through the y-DMA phase; later blocks pace on
            # pool buffer frees.
            mv_blk0 = int(os.environ.get("KCRPS_MV_BLK0", "2"))
            mv_ranges = []
            _g = 0
            while _g < n_pe_groups:
                n_b = mv_blk0 if not mv_ranges else mv_blk
                mv_ranges.append((_g, min(_g + n_b, n_pe_groups)))
                _g += n_b
            blk_of_group = {}
            for bi_, (a_, b_) in enumerate(mv_ranges):
                for g_ in range(a_, b_):
                    blk_of_group[g_] = bi_

            def emit_mv_dma(blk):
                g0, g1 = mv_ranges[blk]
                pool_ = mv0_pool if blk == 0 else mv_pool
                mt = pool_.tile([E + 1, (g1 - g0) * PE_GROUP], f16,
                                tag="mv0" if blk == 0 else "mv")
                eng = {"sync": nc.sync, "scalar": nc.scalar}[
                    os.environ.get("KCRPS_MV_ENG", "scalar")]
                eng.dma_start(
                    out=mt[:],
                    in_=mv.ap()[:, g0 * PE_GROUP:g1 * PE_GROUP])
                mv_tiles[blk] = (mt, g0)

            def emit_pe_groups(n):
                for _ in range(n):
                    g = pe_state["next"]
                    if g >= n_pe_groups:
                        return
                    pe_state["next"] += 1
                    blk = blk_of_group[g]
                    assert blk in mv_tiles
                    mt, g0 = mv_tiles[blk]
                    off = (g - g0) * PE_GROUP
                    pt = ps_pool.tile([PART, PE_GROUP], f32, tag="ps")
                    if g == 0:
                        # warmup matmuls on the tiny weights tile, into the
                        # first group's PSUM (overwritten by the real
                        # start=True matmuls): ramps the PE p-state during
                        # the pipeline fill without claiming a PSUM buffer.
                        for _ in range(int(os.environ.get(
                                "KCRPS_PE_WARM", "0"))):
                            nc.tensor.matmul(out=pt[:, 0:PART], lhsT=wt[:],
                                             rhs=wt[:], start=True,
                                             stop=True)
                    for q in range(PE_GROUP // 512):
                        nc.tensor.matmul(
                            out=pt[:, q * 512:(q + 1) * 512],
                            lhsT=wt[:],
                            rhs=mt[:, off + q * 512:off + (q + 1) * 512],
                            start=True, stop=True)
                    nc.scalar.activation(
                        out=pt[:], in_=pt[:],
                        func=mybir.ActivationFunctionType.Abs,
                        accum_out=acc[:, pe_state["col"]:pe_state["col"] + 1])
                    pe_state["col"] += 1

            # ---- DMA schedule: interleave gps and sort chunks so both
            # GPSIMD and the DVE sort start early.
            gps_off = []
            off = 0
            for w in gps_ws:
                gps_off.append(off)
                off += w
            sort_off = []
            for w in sort_ws:
                sort_off.append(off)
                off += w
            gps_tiles = [None] * len(gps_ws)
            sort_tiles = [None] * len(sort_ws)
            # build the DMA op list: sort chunks are split into two
            # 8-plane halves so the first sort layer can start after h1.
            dma_ops = []            # (kind, i, half)
            omode = os.environ.get("KCRPS_DMA_ORDER", "gs")
            gs = [("g", i, None) for i in range(len(gps_ws))]
            ss = []
            for i in range(len(sort_ws)):
                ss += [("s", i, 0), ("s", i, 1)]
            if omode == "sg":       # all sort halves, then gps
                dma_ops = ss + gs
            elif omode == "sA":     # h1, g0, h2, g1, ...
                dma_ops = []
                pool_ = ss + gs
                a, b = ss, gs
                while a or b:
                    if a:
                        dma_ops.append(a.pop(0))
                    if b:
                        dma_ops.append(b.pop(0))
            else:                   # "gs": g0, h1, h2, g1, ...
                a, b = gs, ss
                while a or b:
                    if a:
                        dma_ops.append(a.pop(0))
                    if b:
                        dma_ops.append(b.pop(0))
                    if b:
                        dma_ops.append(b.pop(0))
            if pe_w:
                nc.sync.dma_start(out=wt[:], in_=wd.ap())
            th_early = os.environ.get("KCRPS_TH_EARLY", "1") == "1"
            th_pos = int(os.environ.get("KCRPS_TH_POS", "1"))
            if th_early:
                dma_ops.insert(th_pos, ("t", 0, None))
            if pe_w:
                # early mv blocks feed the first PE groups; the rest are
                # issued after the y DMAs (all on the sync queue, so a
                # waiting mv DMA never blocks compute issue on ACT/DVE).
                mv_pre = [int(x) for x in os.environ.get(
                    "KCRPS_MV_PRE", "0,2,4").split(",") if x.strip()]
                for bi_, pos in enumerate(mv_pre):
                    dma_ops.insert(min(pos, len(dma_ops)), ("m", bi_, None))
            for kind, i, half in dma_ops:
                if kind == "t":
                    nc.sync.dma_start(out=th[:], in_=t.ap())
                    continue
                if kind == "m":
                    emit_mv_dma(i)
                    continue
                if kind == "g":
                    w, o = gps_ws[i], gps_off[i]
                    if gps_tiles[i] is None:
                        yt = y_pool.tile([PART, E * w], f16, tag="ygps")
                        gps_tiles[i] = (yt, w, o)
                    yt = gps_tiles[i][0]
                    nc.sync.dma_start(
                        out=yt[:], in_=y.ap()[:, o * E:(o + w) * E])
                else:
                    w, o = sort_ws[i], sort_off[i]
                    if sort_tiles[i] is None:
                        yt = ys_pool.tile([PART, E * w], f16, tag="ysort")
                        sort_tiles[i] = (yt, w, o)
                    yt = sort_tiles[i][0]
                    h = E // 2 * w
                    if half == 0:
                        nc.sync.dma_start(
                            out=yt[:, 0:h], in_=y.ap()[:, o * E:o * E + h])
                    else:
                        nc.sync.dma_start(
                            out=yt[:, h:2 * h],
                            in_=y.ap()[:, o * E + h:(o + w) * E])
            if not th_early:
                nc.sync.dma_start(out=th[:], in_=t.ap())
            ydt = None
            if pe_w:
                ydt = ys_pool.tile([PART, n_drop * pe_w], f16, tag="ydrop")
                nc.sync.dma_start(
                    out=ydt[:], in_=y.ap()[:, sbuf_cols * E:y_cols])
                # remaining mv blocks, paced by mv_pool buffer frees; they
                # only ever block the final acc DMA behind them.
                for blk in range(len(mv_ranges)):
                    if blk not in mv_tiles:
                        emit_mv_dma(blk)

            # ---- GPSIMD: all pair diffs of gps chunks ----------------------
            pb_tiles = []
            for yt, w, off0 in gps_tiles:
                pbt = pb_pool.tile([PART, 120 * w], f16, tag="pb")
                cur = 0
                for d in range(1, E):
                    n = E - d
                    nc.gpsimd.tensor_tensor(
                        pbt[:, cur * w:(cur + n) * w],
                        yt[:, 0:n * w],
                        yt[:, d * w:(d + n) * w],
                        AluOpType.subtract)
                    cur += n
                pb_tiles.append((pbt, yt, w, off0))

            # ---- DVE program -----------------------------------------------
            # interleave: sort layers (bulk), gps relu/mae groups (as GPSIMD
            # output becomes ready), drop rows, PE groups stream on ACT.
            def grid_view(tile_ap, grid, w):
                if grid == "16":
                    return tile_ap.rearrange("p (e f) -> p e f", f=w)
                a = {"2x8": 2, "4x4": 4, "8x2": 8}[grid]
                return tile_ap.rearrange("p (a b f) -> p a b f", a=a, f=w)

            def emit_sort(yt, w):
                """Batcher sort of the 16 e-planes of yt using the
                scattered-location plan (no passthrough copies): compared
                planes write to the opposite buffer, untouched planes stay
                put.  Returns (ta, tb); sorted plane k lives in
                bufs[_B_FINAL[k]] slot k."""
                ta = st_pool.tile([PART, E * w], f16, tag="sa")
                tb = st_pool.tile([PART, E * w], f16, tag="sb")
                bufs = (ta, tb)
                yv = grid_view(yt[:], "16", w)
                av = grid_view(ta[:], "16", w)
                # layer 0 reads yt in two plane halves (split DMA), all
                # outputs to A
                for lo, hi in ((0, 8), (8, 16)):
                    i_h = yv[:, lo:hi:2, :]
                    j_h = yv[:, lo + 1:hi:2, :]
                    nc.vector.tensor_tensor(
                        av[:, lo:hi:2, :], i_h, j_h, AluOpType.min)
                    nc.vector.tensor_tensor(
                        av[:, lo + 1:hi:2, :], i_h, j_h, AluOpType.max)

                def vw(bufi, expr):
                    g, osl, isl = expr
                    gv = grid_view(bufs[bufi][:], g, w)
                    if g == "16":
                        return gv[:, osl, :]
                    return gv[:, osl, isl, :]

                for ops in _SCATTER_OPS:
                    for bi, bj, ei, ej, pl, jpl in ops:
                        i_in = vw(bi, ei)
                        j_in = vw(bj, ej)
                        nc.vector.tensor_tensor(
                            vw(1 - bi, ei), i_in, j_in, AluOpType.min)
                        nc.vector.tensor_tensor(
                            vw(1 - bj, ej), i_in, j_in, AluOpType.max)
                return bufs

            def emit_drop():
                dt_ = dr_pool.tile([PART, len(PE_DROP) * pe_w], f16,
                                   tag="dr")
                emit = [
                    (2, 0, 3),   # d=12: planes idx 0..1 vs 3..4
                    (3, 0, 4),   # d=13: idx 0..2 vs 4..6
                    (2, 0, 5),   # d=14: idx 0..1 vs 5..6
                    (1, 0, 6),   # d=15: idx 0 vs 6
                ]
                cur = 0
                for r, i0, i1 in emit:
                    nc.vector.tensor_tensor(
                        dt_[:, cur * pe_w:(cur + r) * pe_w],
                        ydt[:, i0 * pe_w:(i0 + r) * pe_w],
                        ydt[:, i1 * pe_w:(i1 + r) * pe_w],
                        AluOpType.max)
                    cur += r
                nc.vector.tensor_scalar(
                    out=dt_[:], in0=dt_[:], scalar1=0.0, scalar2=0.0,
                    op0=AluOpType.bypass, op1=AluOpType.add,
                    accum_out=acc[:, drop_col:drop_col + 1])

            # --- interleaved emission --------------------------------------
            # Column order must match host decode: per gps chunk
            # [relu x relu_g, gmae], then per sort chunk [coef x16, smae];
            # emission order differs, so allocate columns up-front.
            col_map = {}
            c = 0
            for gi in range(len(gps_ws)):
                for g in range(relu_g):
                    col_map[("gpsrelu", gi, g)] = c
                    c += 1
                col_map[("gmae", gi)] = c
                c += 1
            for si in range(len(sort_ws)):
                for k in range(E):
                    col_map[("coef", si, k)] = c
                    c += 1
                col_map[("smae", si)] = c
                c += 1
            assert c == pe_state["col"]

            def gps_consume(gi, g, scratch=None):
                # The elementwise relu output is unused (only accum_out
                # matters).  Writing it into the sort scratch region gives
                # the op a WAR hazard against the final sort layers, which
                # pins it late in the DVE stream -- the tile scheduler's
                # internal cost model underestimates GPSIMD time by ~2.4x
                # and otherwise hoists these between early sort layers,
                # head-of-line blocking the DVE for many microseconds.
                pbt, yt, w, off0 = pb_tiles[gi]
                g0, g1 = pb_groups[g]
                cc = col_map[("gpsrelu", gi, g)]
                n = (g1 - g0) * w
                out_ap = (scratch[:, 0:n] if scratch is not None
                          else pbt[:, g0 * w:g1 * w])
                nc.vector.tensor_scalar(
                    out=out_ap, in0=pbt[:, g0 * w:g1 * w],
                    scalar1=0.0, scalar2=0.0,
                    op0=AluOpType.max, op1=AluOpType.add,
                    accum_out=acc[:, cc:cc + 1])

            def gps_mae(gi):
                pbt, yt, w, off0 = pb_tiles[gi]
                mt = sc_pool.tile([PART, E * w], f16, tag="gmae")
                yv = yt[:].rearrange("p (e f) -> p e f", e=E)
                tb = (th[:, pe_w + off0:pe_w + off0 + w]
                      .unsqueeze(1).broadcast_to([PART, E, w]))
                mv_ = mt[:].rearrange("p (e f) -> p e f", e=E)
                nc.vector.tensor_tensor(mv_[:, :, :], yv[:, :, :], tb,
                                        AluOpType.max)
                cc = col_map[("gmae", gi)]
                nc.vector.tensor_scalar(
                    out=mt[:], in0=mt[:], scalar1=0.0, scalar2=0.0,
                    op0=AluOpType.bypass, op1=AluOpType.add,
                    accum_out=acc[:, cc:cc + 1])

            def sort_coef(si, bufs, w):
                for k in range(E):
                    sv = bufs[_B_FINAL[k]][:].rearrange(
                        "p (e f) -> p e f", e=E)
                    cc = col_map[("coef", si, k)]
                    nc.vector.tensor_scalar(
                        out=sv[:, k, :], in0=sv[:, k, :],
                        scalar1=float(2 * k - (E - 1)), scalar2=0.0,
                        op0=AluOpType.mult, op1=AluOpType.add,
                        accum_out=acc[:, cc:cc + 1])

            def sort_mae(si, yt, w, off0):
                # mae is permutation-invariant: read the original
                # (unsorted) planes straight from the DMA tile, in place
                # (yt is dead after layer 0 + this).
                yv = yt[:].rearrange("p (e f) -> p e f", e=E)
                tb = (th[:, pe_w + off0:pe_w + off0 + w]
                      .unsqueeze(1).broadcast_to([PART, E, w]))
                nc.vector.tensor_tensor(yv[:, :, :], yv[:, :, :], tb,
                                        AluOpType.max)
                cc = col_map[("smae", si)]
                nc.vector.tensor_scalar(
                    out=yt[:], in0=yt[:], scalar1=0.0,
                    scalar2=0.0,
                    op0=AluOpType.bypass, op1=AluOpType.add,
                    accum_out=acc[:, cc:cc + 1])

            # emission: interleave DVE work so it rarely stalls on GPSIMD,
            # and spread PE-group emission so mv DMA keeps ahead of PE.
            ngps = len(gps_tiles)
            assert len(sort_tiles) >= 1
            # kick a first batch of PE groups so ACT starts early
            emit_pe_groups(int(os.environ.get("KCRPS_EARLY_PE", "4")))
            if os.environ.get("KCRPS_GMAE0_EARLY", "1") == "1":
                # gps chunk 0's mae needs only ygps0+th (land early): fills
                # the DVE idle window before the first sort DMA completes
                gps_mae(0)

            # sort chunks at high priority: the scheduler slots gps
            # consumers into DVE idle moments but prefers sort work the
            # moment its data lands.
            scratches = []
            for si, (yts, ws, offs) in enumerate(sort_tiles):
                with tc.high_priority():
                    bufs = emit_sort(yts, ws)
                    sort_mae(si, yts, ws, offs)
                    sort_coef(si, bufs, ws)
                scratches.append(bufs[0])
                emit_pe_groups(4)
            if pe_w:
                emit_drop()

            # gps consumers last; their dummy outputs write into the final
            # sort scratch to pin them after the sort (see gps_consume).
            pin = scratches[-1]
            for gi in range(ngps):
                for g in range(relu_g):
                    gps_consume(gi, g, scratch=pin)
                    emit_pe_groups(2)
                if gi > 0 or os.environ.get("KCRPS_GMAE0_EARLY", "1") != "1":
                    gps_mae(gi)

            emit_pe_groups(n_pe_groups - pe_state["next"])

            out_eng = {"sync": nc.sync, "scalar": nc.scalar,
                       "gpsimd": nc.gpsimd}[
                os.environ.get("KCRPS_OUT_ENG", "sync")]
            out_eng.dma_start(out=out.ap(), in_=acc[:])
    nc.compile()
    nc._kcrps_meta = (pe_w, tuple(gps_ws), tuple(sort_ws), relu_g, ncol)
    return nc


def kernel(y_pred, y_target, weights, scale):
    global LAST_EXEC_NS, LAST_NC
    from concourse.bass_utils import run_bass_kernel_spmd

    pe_w = _pe_w()
    gps_ws = _gps_ws()
    sort_ws = _sort_ws()
    relu_g = int(os.environ.get("KCRPS_RELU_GROUPS", "3"))
    key = ("v2", pe_w, tuple(gps_ws), tuple(sort_ws), relu_g, PE_GROUP)
    if key not in _CACHE:
        _CACHE[key] = _build_nc(pe_w, gps_ws, sort_ws)
    nc = _CACHE[key]
    LAST_NC = nc

    y_pred = np.asarray(y_pred, dtype=np.float32)
    y_target = np.asarray(y_target, dtype=np.float32)
    weights = np.asarray(weights, dtype=np.float32)
    scale = np.asarray(scale, dtype=np.float32)

    ghat = (scale[None, :, None] * weights[None, None, :])     # (1, V, P)
    yh = (y_pred * ghat[..., None]).astype(np.float16)         # (B, V, P, E)
    th = (y_target * ghat).astype(np.float16)                  # (B, V, P)

    n_drop = len(PE_DROP_PLANES)
    sbuf_cols = sum(gps_ws) + sum(sort_ws)
    gps_tot = sum(gps_ws)

    if pe_w:
        W = np.zeros((E + 1, PART), np.float16)
        for m, (d, i) in enumerate(PE_PAIRS):
            W[i, m] = 1.0
            W[i + d, m] = -1.0
        for k in range(E):
            W[E, 112 + k] = 1.0
            W[k, 112 + k] = -1.0

    in_maps = []
    C_gps = np.zeros(E, np.float64)
    C_sbuf = np.zeros(E, np.float64)
    C_pe = np.zeros(E, np.float64)
    T1_sbuf = 0.0
    for c in range(NCORES):
        sl = slice(c * PC, (c + 1) * PC)
        arr = yh[:, :, sl, :].reshape(PART, FREE, E)
        tharr = th[:, :, sl].reshape(PART, FREE)
        segs = []
        off = pe_w
        for w in list(gps_ws) + list(sort_ws):
            seg = arr[:, off:off + w, :].transpose(0, 2, 1)    # (PART, E, w)
            segs.append(seg.reshape(PART, E * w))
            off += w
        imap = {}
        if pe_w:
            dseg = (arr[:, 0:pe_w, :][:, :, PE_DROP_PLANES]
                    .transpose(0, 2, 1).reshape(PART, n_drop * pe_w))
            segs.append(dseg)
            mvy = arr[:, 0:pe_w, :].reshape(PART * pe_w, E).T  # (E, S)
            mvt = tharr[:, 0:pe_w].reshape(1, PART * pe_w)
            imap["mv"] = np.ascontiguousarray(
                np.concatenate([mvy, mvt], axis=0).astype(np.float16))
            imap["wm"] = W
            C_pe += arr[:, 0:pe_w, :].astype(np.float64).sum(axis=(0, 1))
        imap["y"] = np.ascontiguousarray(np.concatenate(segs, axis=1))
        imap["t"] = np.ascontiguousarray(tharr)
        in_maps.append(imap)
        C_gps += (arr[:, pe_w:pe_w + gps_tot, :]
                  .astype(np.float64).sum(axis=(0, 1)))
        C_sbuf += arr[:, pe_w:, :].astype(np.float64).sum(axis=(0, 1))
        T1_sbuf += tharr[:, pe_w:].astype(np.float64).sum()

    res = run_bass_kernel_spmd(
        nc, in_maps, core_ids=list(range(NCORES)), trace=False)
    LAST_EXEC_NS = res.exec_time_ns

    n_pe_groups = PART * pe_w // PE_GROUP if pe_w else 0
    R_relu = M_gmae = M_smae = 0.0
    PAIR_sort = 0.0
    A_abs = A_mae = M_drop = 0.0
    for c in range(NCORES):
        a = res.results[c]["acc"].astype(np.float64)
        cc = 0
        for gi in range(len(gps_ws)):
            for g in range(relu_g):
                R_relu += a[:, cc].sum()
                cc += 1
            M_gmae += a[:, cc].sum()
            cc += 1
        for si in range(len(sort_ws)):
            for k in range(E):
                PAIR_sort += a[:, cc].sum()
                cc += 1
            M_smae += a[:, cc].sum()
            cc += 1
        if pe_w:
            pe_cols = a[:, cc:cc + n_pe_groups]
            A_abs += pe_cols[0:112, :].sum()       # matrix pair rows
            A_mae += pe_cols[112:128, :].sum()     # matrix mae rows
            M_drop += a[:, cc + n_pe_groups].sum()  # dropped pair rows

    # linear corrections (exact, fp64, from fp16 inputs)
    L_gps = 0.0          # sum over all (d,i) pairs of (C_i - C_{i+d})
    for d in range(1, E):
        for i in range(E - d):
            L_gps += C_gps[i] - C_gps[i + d]
    L_drop = 0.0
    for d, i in PE_DROP:
        L_drop += C_pe[i] + C_pe[i + d]

    PAIR_total = (A_abs + PAIR_sort
                  + 2.0 * R_relu - L_gps
                  + 2.0 * M_drop - L_drop)
    MAE_total = (A_mae + 2.0 * (M_gmae + M_smae)
                 - E * T1_sbuf - C_sbuf.sum())
    npoints = weights.astype(np.float64).sum()
    result = (MAE_total / E - PAIR_total / (E * E)) / (npoints * B)
    return np.float32(result)
